# revision 26
# baseline (speedup 1.0000x reference)
"""Distributed Bass kernel for causal multi-head attention with RoPE.

Problem: B=2, S=2048, D=2048, H=16, HD=128 (nn_Attention_85315230368481).

Sharding: tensor-parallel over heads. Core c owns heads {2c, 2c+1} and
computes Q/K/V projections for those heads over the FULL sequence
(4096 rows = both batches), applies RoPE, then causal attention for its
2 heads (skipping fully-masked 128x512 key/query blocks), and finally
the output projection for its 512-row slice of the output. The per-head
attention outputs are exchanged with a single AllToAll per head (each
core sends its heads' columns split by destination row-slice and
receives every head's values for its own rows) -- 8x less fabric
traffic than an AllGather of K/V or of attention outputs.

Layout tricks:
 - x arrives transposed ([D, 4096]); Q^T/K^T come out of the PE as
   [head_dim, rows] and V in natural [rows, head_dim], so no on-chip
   transposes are needed anywhere.
 - Wq/Wk columns are permuted per head (even dims then odd dims) so
   RoPE works on contiguous partition halves; scores are invariant to
   the permutation since both Q and K use it.
 - Scores are computed transposed ([keys, queries]); softmax
   denominators accumulate on the vector engine (one add per exp tile)
   with a single ones-matmul partition-reduction per query tile.
 - exp is computed shifted (exp(s/sqrt(hd) - 5)) so probabilities and
   denominators stay in fp16 range; softmax is shift-invariant.
 - Diagonal 128x512 blocks use one of 4 precomputed relative causal
   masks; fully-masked blocks are skipped outright.
 - Wo rows are pre-permuted to AllToAll arrival order so the output
   projection consumes exchange chunks directly.
 - All matmuls and element-wise ops in fp16 (fp32 accumulation in
   PSUM; fp32 softmax denominator reduction) -- full PE rate and 2-4x
   DVE rate vs fp32.
"""

import sys

import numpy as np

if "/opt/trn_rl_repo" not in sys.path:
    sys.path.insert(0, "/opt/trn_rl_repo")

B, S, D, H = 2, 2048, 2048, 16
HD = D // H            # 128
NCORES = 8
HPC = H // NCORES      # 2 heads per core
ROWS = B * S           # 4096 rows total (both batches)
ORON = ROWS // NCORES  # 512 output rows per core
DCH = D // 128         # 16 contraction chunks
NQT = S // 512         # 4 query tiles of 512 per batch
NRT = ROWS // 512      # 8 row tiles of 512 (projection)
SCALE = 1.0 / float(np.sqrt(HD))
ESHIFT = -5.0          # exp(s*SCALE + ESHIFT): keeps sums in fp16 range
F16 = np.float16

_GRAPH = None
_TRACE = False
_LAST_EXEC_NS = None
_LAST_RES = None


def _build_graph():
    import concourse.mybir as mybir
    from concourse import bacc, tile

    f32 = mybir.dt.float32
    f16 = mybir.dt.float16
    Exp = mybir.ActivationFunctionType.Exp

    nc = bacc.Bacc("TRN2", target_bir_lowering=False, num_devices=NCORES)

    xT = nc.declare_dram_parameter("xT", [D, ROWS], f16, isOutput=False)
    wq = nc.declare_dram_parameter("wq", [D, HPC * HD], f16, isOutput=False)
    wk = nc.declare_dram_parameter("wk", [D, HPC * HD], f16, isOutput=False)
    wv = nc.declare_dram_parameter("wv", [D, HPC * HD], f16, isOutput=False)
    wo = nc.declare_dram_parameter("wo", [D, D], f16, isOutput=False)
    # cos/sin duplicated across both partition halves (DVE same-base rule)
    cosT = nc.declare_dram_parameter("cosT", [HD, S], f16, isOutput=False)
    sinT = nc.declare_dram_parameter("sinT", [HD, S], f16, isOutput=False)
    # 4 relative diagonal-mask blocks [128, 512] (applied on the PE via
    # an accumulating identity-matmul; -30000 is "-inf" for fp16)
    maskd = nc.declare_dram_parameter("maskd", [128, 4 * 512], f16, isOutput=False)
    onesh = nc.declare_dram_parameter("onesh", [128, 128], f16, isOutput=False)
    identh = nc.declare_dram_parameter("identh", [128, 128], f16, isOutput=False)
    out = nc.declare_dram_parameter("out", [ORON, D], f32, isOutput=True)

    with nc.allow_low_precision(reason="fp16 matmul/vector; fp32 accumulate"), \
         tile.TileContext(nc) as tc:
        with (
            tc.tile_pool(name="dram", bufs=1, space="DRAM") as dramp,
            tc.tile_pool(name="resid", bufs=1) as resid,
        ):
            a2a_in = [dramp.tile([NCORES * HD, ORON], f16, name=f"a2ain{h}")
                      for h in range(HPC)]
            a2a_out = [dramp.tile([NCORES * HD, ORON], f16, name=f"a2aout{h}")
                       for h in range(HPC)]

            # long-lived SBUF tensors
            cos_sb = resid.tile([128, S], f16)
            sin_sb = resid.tile([128, S], f16)
            nc.sync.dma_start(out=cos_sb[:], in_=cosT[:, :])
            nc.sync.dma_start(out=sin_sb[:], in_=sinT[:, :])
            mask_sb = resid.tile([128, 4 * 512], f16)
            nc.sync.dma_start(out=mask_sb[:], in_=maskd[:, :])
            ones_sb = resid.tile([128, 128], f16)
            nc.sync.dma_start(out=ones_sb[:], in_=onesh[:, :])
            ident_sb = resid.tile([128, 128], f16)
            nc.sync.dma_start(out=ident_sb[:], in_=identh[:, :])

            eshift_sb = resid.tile([128, 1], f32)
            nc.vector.memset(eshift_sb[:], ESHIFT)

            qT = resid.tile([128, HPC * ROWS], f16)   # rope'd Q^T per head
            kT = resid.tile([128, HPC * ROWS], f16)   # rope'd K^T per head
            v_sb = resid.tile([128, (ROWS // 128) * HPC * HD], f16)
            attT = resid.tile([128, HPC * ROWS], f16)  # attention out^T per head

            # ---- Q/K/V projections (+ RoPE on Q,K) ----
            with (
                tc.tile_pool(name="wsb", bufs=1) as wsb,
                tc.tile_pool(name="xstream", bufs=4) as xpool,
                tc.tile_pool(name="qkps", bufs=1, space="PSUM") as qkps,
                tc.tile_pool(name="vps", bufs=1, space="PSUM") as vpsp,
                tc.tile_pool(name="drains", bufs=2) as drainp,
                tc.tile_pool(name="ropetmp", bufs=2) as ropep,
            ):
                wq_sb = wsb.tile([128, DCH * HPC * HD], f16)
                wk_sb = wsb.tile([128, DCH * HPC * HD], f16)
                wv_sb = wsb.tile([128, DCH * HPC * HD], f16)
                # interleave by chunk so the first matmuls start early
                for d in range(DCH):
                    for t_sb, t_dram in ((wq_sb, wq), (wk_sb, wk), (wv_sb, wv)):
                        nc.sync.dma_start(
                            out=t_sb[:, d * HPC * HD:(d + 1) * HPC * HD],
                            in_=t_dram[d * 128:(d + 1) * 128, :],
                        )

                def rope(dst, dst_col, src, s0):
                    # src: [128, 512] sbuf fp16; rows 0:64 even dims, 64:128 odd
                    te = src[0:64, :]
                    to = src[64:128, :]
                    cl = cos_sb[0:64, s0:s0 + 512]
                    ch = cos_sb[64:128, s0:s0 + 512]
                    sl = sin_sb[0:64, s0:s0 + 512]
                    sh = sin_sb[64:128, s0:s0 + 512]
                    t1 = ropep.tile([64, 512], f16, tag="r1")
                    t2 = ropep.tile([64, 512], f16, tag="r2")
                    nc.vector.tensor_mul(t1[:], te, cl)
                    nc.vector.tensor_mul(t2[:], to, sh)
                    nc.vector.tensor_sub(dst[0:64, dst_col:dst_col + 512],
                                         t1[:], t2[:])
                    t3 = ropep.tile([64, 512], f16, tag="r3")
                    t4 = ropep.tile([64, 512], f16, tag="r4")
                    nc.vector.tensor_mul(t3[:], te, sl)
                    nc.vector.tensor_mul(t4[:], to, ch)
                    nc.vector.tensor_add(dst[64:128, dst_col:dst_col + 512],
                                         t3[:], t4[:])

                for rt in range(NRT):
                    s0 = (rt % NQT) * 512
                    qk = [qkps.tile([128, 512], f32, tag=f"qk{i}",
                                    name="qktile") for i in range(4)]
                    vp = [vpsp.tile([128, 256], f32, tag=f"vp{i}",
                                    name="vptile") for i in range(4)]
                    for d in range(DCH):
                        x_t = xpool.tile([128, 512], f16, tag="xt")
                        nc.gpsimd.dma_start(
                            out=x_t[:],
                            in_=xT[d * 128:(d + 1) * 128,
                                   rt * 512:(rt + 1) * 512],
                        )
                        first, last = d == 0, d == DCH - 1
                        c0 = d * HPC * HD
                        for hh in range(HPC):
                            nc.tensor.matmul(
                                qk[hh][:],
                                lhsT=wq_sb[:, c0 + hh * HD:c0 + (hh + 1) * HD],
                                rhs=x_t[:], start=first, stop=last,
                            )
                            nc.tensor.matmul(
                                qk[2 + hh][:],
                                lhsT=wk_sb[:, c0 + hh * HD:c0 + (hh + 1) * HD],
                                rhs=x_t[:], start=first, stop=last,
                            )
                        for sub in range(4):
                            nc.tensor.matmul(
                                vp[sub][:],
                                lhsT=x_t[:, sub * 128:(sub + 1) * 128],
                                rhs=wv_sb[:, c0:c0 + HPC * HD],
                                start=first, stop=last,
                            )
                    # fast drain: psum -> sbuf fp16 copies, then RoPE off-psum
                    for i in range(4):
                        qksb = drainp.tile([128, 512], f16, tag=f"dr{i}")
                        nc.scalar.copy(qksb[:], qk[i][:])
                        dst = qT if i < 2 else kT
                        rope(dst, (i % 2) * ROWS + rt * 512, qksb, s0)
                    for sub in range(4):
                        gc = rt * 4 + sub
                        nc.vector.tensor_copy(
                            v_sb[:, gc * 256:(gc + 1) * 256], vp[sub][:])

            # ---- Attention per (head, batch), causal-skipped ----
            with tc.tile_pool(name="wop", bufs=1) as wop:
                # Wo resident, chunk-major (prefetched during attention)
                wo_sb = wop.tile([128, DCH * D], f16)
                for k in range(DCH):
                    nc.sync.dma_start(
                        out=wo_sb[:, k * D:(k + 1) * D],
                        in_=wo[k * 128:(k + 1) * 128, :],
                    )
                with (
                    tc.tile_pool(name="scps", bufs=4, space="PSUM") as scps,
                    tc.tile_pool(name="attps", bufs=1, space="PSUM") as attps,
                    tc.tile_pool(name="extiles", bufs=6) as expool,
                    tc.tile_pool(name="esum", bufs=2) as esump,
                    tc.tile_pool(name="araw", bufs=2) as arawp,
                    tc.tile_pool(name="tmp", bufs=2) as tmpp,
                ):
                    # deferred work from the previous head-batch, issued
                    # early in the next one so it never stalls an engine
                    pend_norm = []
                    pend_a2a = []

                    def flush_deferred():
                        for fn in pend_norm:
                            fn()
                        pend_norm.clear()
                        for fn in pend_a2a:
                            fn()
                        pend_a2a.clear()

                    for h in range(HPC):
                        for b in range(B):
                            kcol = h * ROWS + b * S
                            att = [attps.tile([128, 512], f32, tag=f"att{qt}",
                                              name="atttile")
                                   for qt in range(NQT)]
                            esum = [esump.tile([128, 512], f16, tag=f"es{qt}",
                                               name="esumtile")
                                    for qt in range(NQT)]
                            pend = []

                            def flush_attv(limit, att=att, h=h, b=b):
                                while len(pend) > limit:
                                    qt2, kc2, ex2 = pend.pop(0)
                                    gc2 = (b * DCH + kc2) * HPC * HD + h * HD
                                    nc.tensor.matmul(
                                        att[qt2][:],
                                        lhsT=v_sb[:, gc2:gc2 + HD],
                                        rhs=ex2[:],
                                        start=(kc2 == 0),
                                        stop=(kc2 == 4 * qt2 + 3),
                                    )

                            for kc in range(DCH):
                                for qt in range(kc // 4, NQT):
                                    diag = kc // 4 == qt
                                    sc = scps.tile([128, 512], f32, tag="sc")
                                    nc.tensor.matmul(
                                        sc[:],
                                        lhsT=kT[:, kcol + kc * 128:
                                                kcol + (kc + 1) * 128],
                                        rhs=qT[:, kcol + qt * 512:
                                               kcol + (qt + 1) * 512],
                                        start=True, stop=not diag,
                                    )
                                    if diag:  # mask on the PE: sc += I.T @ M
                                        kk = kc % 4
                                        nc.tensor.matmul(
                                            sc[:], lhsT=ident_sb[:],
                                            rhs=mask_sb[:, kk * 512:
                                                        (kk + 1) * 512],
                                            start=False, stop=True,
                                        )
                                    ex = expool.tile([128, 512], f16, tag="ex")
                                    nc.scalar.activation(ex[:], sc[:], Exp,
                                                         bias=eshift_sb[:],
                                                         scale=SCALE)
                                    if kc == 0:
                                        nc.vector.tensor_copy(esum[qt][:], ex[:])
                                    else:
                                        nc.vector.tensor_add(
                                            esum[qt][:], esum[qt][:], ex[:])
                                    pend.append((qt, kc, ex))
                                    flush_attv(3)
                                if kc == 1:
                                    flush_deferred()
                            flush_attv(0)
                            # drain att psum to sbuf fp16 (frees banks), then
                            # defer den/recip/normalize into the next batch
                            araw = [arawp.tile([128, 512], f16, tag=f"ar{qt}",
                                               name="arawtile")
                                    for qt in range(NQT)]
                            for qt in range(NQT):
                                nc.vector.tensor_copy(araw[qt][:], att[qt][:])

                            def norm(h=h, b=b, kcol=kcol, araw=araw,
                                     esum=esum):
                                for qt in range(NQT):
                                    den = scps.tile([128, 512], f32, tag="sc")
                                    nc.tensor.matmul(
                                        den[0:1, :], lhsT=ones_sb[:, 0:1],
                                        rhs=esum[qt][:], start=True, stop=True,
                                    )
                                    rcp = tmpp.tile([1, 512], f16, tag="rcp")
                                    nc.vector.reciprocal(rcp[:], den[0:1, :])
                                    rb = scps.tile([128, 512], f32, tag="sc")
                                    nc.tensor.matmul(
                                        rb[:], lhsT=ones_sb[0:1, :],
                                        rhs=rcp[:], start=True, stop=True,
                                    )
                                    rbs = tmpp.tile([128, 512], f16, tag="rbs")
                                    nc.vector.tensor_copy(rbs[:], rb[:])
                                    nc.vector.tensor_mul(
                                        attT[:, kcol + qt * 512:
                                             kcol + (qt + 1) * 512],
                                        araw[qt][:], rbs[:],
                                    )

                            pend_norm.append(norm)
                            if b == B - 1:
                                def a2a(h=h):
                                    for dd in range(NCORES):
                                        nc.sync.dma_start(
                                            out=a2a_in[h][dd * 128:
                                                          (dd + 1) * 128, :],
                                            in_=attT[:, h * ROWS + dd * 512:
                                                     h * ROWS +
                                                     (dd + 1) * 512],
                                        )
                                    nc.gpsimd.collective_compute(
                                        "AllToAll",
                                        mybir.AluOpType.bypass,
                                        replica_groups=[list(range(NCORES))],
                                        ins=[a2a_in[h][:].opt()],
                                        outs=[a2a_out[h][:].opt()],
                                    )
                                pend_a2a.append(a2a)
                    flush_deferred()

                # ---- Output projection over exchange chunks ----
                with (
                    tc.tile_pool(name="atile", bufs=1) as atp,
                    tc.tile_pool(name="ops", bufs=2, space="PSUM") as opsp,
                    tc.tile_pool(name="osb", bufs=2) as osbp,
                ):
                    aT = atp.tile([128, DCH * 512], f16)
                    for k in range(DCH):
                        h, r = k // NCORES, k % NCORES
                        nc.gpsimd.dma_start(
                            out=aT[:, k * 512:(k + 1) * 512],
                            in_=a2a_out[h][r * 128:(r + 1) * 128, :],
                        )
                    for m in range(4):
                        ops = opsp.tile([128, D], f32, tag="ops",
                                        name="opstile")
                        for k in range(DCH):
                            for n in range(4):
                                nc.tensor.matmul(
                                    ops[:, n * 512:(n + 1) * 512],
                                    lhsT=aT[:, k * 512 + m * 128:
                                            k * 512 + (m + 1) * 128],
                                    rhs=wo_sb[:, k * D + n * 512:
                                              k * D + (n + 1) * 512],
                                    start=(k == 0), stop=(k == DCH - 1),
                                )
                        osb = osbp.tile([128, D], f32, tag="osb")
                        nc.scalar.copy(osb[:], ops[:])
                        nc.sync.dma_start(
                            out=out[m * 128:(m + 1) * 128, :], in_=osb[:],
                        )

    nc.compile()
    return nc


def _get_graph():
    global _GRAPH
    if _GRAPH is None:
        _GRAPH = _build_graph()
    return _GRAPH


_EVENODD = np.concatenate([np.arange(0, HD, 2), np.arange(1, HD, 2)])


def kernel(x, Wq, Wk, Wv, Wo, freqs_cos, freqs_sin, mask):
    global _LAST_EXEC_NS, _LAST_RES
    from concourse.bass_utils import run_bass_kernel_spmd

    nc = _get_graph()

    x = np.asarray(x, np.float32)
    Wq = np.asarray(Wq, np.float32)
    Wk = np.asarray(Wk, np.float32)
    Wv = np.asarray(Wv, np.float32)
    Wo = np.asarray(Wo, np.float32)

    xTb = np.ascontiguousarray(x.reshape(ROWS, D).T).astype(F16)
    cosf = np.asarray(freqs_cos, np.float32).T.astype(F16)
    sinf = np.asarray(freqs_sin, np.float32).T.astype(F16)
    cosf = np.ascontiguousarray(np.concatenate([cosf, cosf], axis=0))
    sinf = np.ascontiguousarray(np.concatenate([sinf, sinf], axis=0))

    # 4 relative diagonal mask blocks; -30000 acts as -inf after scaling
    i_idx = np.arange(128)[:, None]
    q_idx = np.arange(512)[None, :]
    maskd = np.concatenate(
        [np.where(kk * 128 + i_idx > q_idx, np.float32(-30000.0),
                  np.float32(0.0)) for kk in range(4)], axis=1)
    maskd = np.ascontiguousarray(maskd).astype(F16)
    identh = np.eye(128, dtype=F16)

    # Wo rows permuted to AllToAll arrival order: heads 0,2,..,14,1,3,..,15
    row_order = np.concatenate(
        [np.arange(h * HD, (h + 1) * HD)
         for h in [2 * r for r in range(NCORES)]
         + [2 * r + 1 for r in range(NCORES)]])
    wo_p = np.ascontiguousarray(Wo[row_order, :]).astype(F16)
    ones_h = np.ones((128, 128), F16)

    in_maps = []
    for c in range(NCORES):
        cols_pq = np.concatenate(
            [(2 * c + hh) * HD + _EVENODD for hh in range(HPC)])
        cols_v = np.arange(2 * c * HD, (2 * c + HPC) * HD)
        in_maps.append({
            "xT": xTb,
            "wq": np.ascontiguousarray(Wq[:, cols_pq]).astype(F16),
            "wk": np.ascontiguousarray(Wk[:, cols_pq]).astype(F16),
            "wv": np.ascontiguousarray(Wv[:, cols_v]).astype(F16),
            "wo": wo_p,
            "cosT": cosf, "sinT": sinf,
            "maskd": maskd, "onesh": ones_h, "identh": identh,
        })

    res = run_bass_kernel_spmd(
        nc, in_maps, core_ids=list(range(NCORES)), trace=_TRACE,
    )
    _LAST_EXEC_NS = res.exec_time_ns
    _LAST_RES = res

    outp = np.empty((ROWS, D), np.float32)
    for c in range(NCORES):
        outp[c * ORON:(c + 1) * ORON, :] = res.results[c]["out"]
    return outp.reshape(B, S, D)


# revision 30
# speedup vs baseline: 1.1865x; 1.1865x over previous
"""Distributed Bass kernel for causal multi-head attention with RoPE.

Problem: B=2, S=2048, D=2048, H=16, HD=128 (nn_Attention_85315230368481).

Sharding: tensor-parallel over heads. Core c owns heads {2c, 2c+1} and
computes Q/K/V projections for those heads over the FULL sequence
(4096 rows = both batches), applies RoPE, then causal attention for its
2 heads (skipping fully-masked 128x512 key/query blocks), and finally
the output projection for its 512-row slice of the output. The per-head
attention outputs are exchanged with a single AllToAll per head (each
core sends its heads' columns split by destination row-slice and
receives every head's values for its own rows) -- 8x less fabric
traffic than an AllGather of K/V or of attention outputs.

Layout tricks:
 - x arrives transposed ([D, 4096]); Q^T/K^T come out of the PE as
   [head_dim, rows] and V in natural [rows, head_dim], so no on-chip
   transposes are needed anywhere.
 - Wq/Wk columns are permuted per head (even dims then odd dims) so
   RoPE works on contiguous partition halves; scores are invariant to
   the permutation since both Q and K use it.
 - Scores are computed transposed ([keys, queries]); softmax
   denominators accumulate on the vector engine (one add per exp tile)
   with a single ones-matmul partition-reduction per query tile.
 - exp is computed shifted (exp(s/sqrt(hd) - 5)) so probabilities and
   denominators stay in fp16 range; softmax is shift-invariant.
 - Diagonal 128x512 blocks use one of 4 precomputed relative causal
   masks; fully-masked blocks are skipped outright.
 - Wo rows are pre-permuted to AllToAll arrival order so the output
   projection consumes exchange chunks directly.
 - All matmuls and element-wise ops in fp16 (fp32 accumulation in
   PSUM; fp32 softmax denominator reduction) -- full PE rate and 2-4x
   DVE rate vs fp32.
"""

import sys

import numpy as np

if "/opt/trn_rl_repo" not in sys.path:
    sys.path.insert(0, "/opt/trn_rl_repo")

B, S, D, H = 2, 2048, 2048, 16
HD = D // H            # 128
NCORES = 8
HPC = H // NCORES      # 2 heads per core
ROWS = B * S           # 4096 rows total (both batches)
ORON = ROWS // NCORES  # 512 output rows per core
DCH = D // 128         # 16 contraction chunks
NQT = S // 512         # 4 query tiles of 512 per batch
NRT = ROWS // 512      # 8 row tiles of 512 (projection)
SCALE = 1.0 / float(np.sqrt(HD))
ESHIFT = -5.0          # exp(s*SCALE + ESHIFT): keeps sums in fp16 range
F16 = np.float16

_GRAPH = None
_TRACE = False
_LAST_EXEC_NS = None
_LAST_RES = None


def _build_graph():
    import concourse.mybir as mybir
    from concourse import bacc, tile

    f32 = mybir.dt.float32
    f16 = mybir.dt.float16
    Exp = mybir.ActivationFunctionType.Exp

    nc = bacc.Bacc("TRN2", target_bir_lowering=False, num_devices=NCORES)

    xT = nc.declare_dram_parameter("xT", [D, ROWS], f16, isOutput=False)
    wq = nc.declare_dram_parameter("wq", [D, HPC * HD], f16, isOutput=False)
    wk = nc.declare_dram_parameter("wk", [D, HPC * HD], f16, isOutput=False)
    wv = nc.declare_dram_parameter("wv", [D, HPC * HD], f16, isOutput=False)
    wo = nc.declare_dram_parameter("wo", [D, D], f16, isOutput=False)
    # cos/sin duplicated across both partition halves (DVE same-base rule)
    cosT = nc.declare_dram_parameter("cosT", [HD, S], f16, isOutput=False)
    sinT = nc.declare_dram_parameter("sinT", [HD, S], f16, isOutput=False)
    # 4 relative diagonal-mask blocks [128, 512] (applied on the PE via
    # an accumulating identity-matmul; -30000 is "-inf" for fp16)
    maskd = nc.declare_dram_parameter("maskd", [128, 4 * 512], f16, isOutput=False)
    onesh = nc.declare_dram_parameter("onesh", [128, 128], f16, isOutput=False)
    identh = nc.declare_dram_parameter("identh", [128, 128], f16, isOutput=False)
    out = nc.declare_dram_parameter("out", [ORON, D], f32, isOutput=True)

    with nc.allow_low_precision(reason="fp16 matmul/vector; fp32 accumulate"), \
         tile.TileContext(nc) as tc:
        with (
            tc.tile_pool(name="dram", bufs=1, space="DRAM") as dramp,
            tc.tile_pool(name="resid", bufs=1) as resid,
        ):
            a2a_in = [dramp.tile([NCORES * HD, ORON], f16, name=f"a2ain{h}")
                      for h in range(HPC)]
            a2a_out = [dramp.tile([NCORES * HD, ORON], f16, name=f"a2aout{h}")
                       for h in range(HPC)]

            # long-lived SBUF tensors
            cos_sb = resid.tile([128, S], f16)
            sin_sb = resid.tile([128, S], f16)
            nc.sync.dma_start(out=cos_sb[:], in_=cosT[:, :])
            nc.sync.dma_start(out=sin_sb[:], in_=sinT[:, :])
            mask_sb = resid.tile([128, 4 * 512], f16)
            nc.sync.dma_start(out=mask_sb[:], in_=maskd[:, :])
            ones_sb = resid.tile([128, 128], f16)
            nc.sync.dma_start(out=ones_sb[:], in_=onesh[:, :])
            ident_sb = resid.tile([128, 128], f16)
            nc.sync.dma_start(out=ident_sb[:], in_=identh[:, :])

            eshift_sb = resid.tile([128, 1], f32)
            nc.vector.memset(eshift_sb[:], ESHIFT)

            qT = resid.tile([128, HPC * ROWS], f16)   # rope'd Q^T per head
            kT = resid.tile([128, HPC * ROWS], f16)   # rope'd K^T per head
            v_sb = resid.tile([128, (ROWS // 128) * HPC * HD], f16)
            attT = resid.tile([128, HPC * ROWS], f16)  # attention out^T per head

            # ---- Q/K/V projections (+ RoPE on Q,K) ----
            with (
                tc.tile_pool(name="wsb", bufs=1) as wsb,
                tc.tile_pool(name="xstream", bufs=4) as xpool,
                tc.tile_pool(name="qkps", bufs=1, space="PSUM") as qkps,
                tc.tile_pool(name="vps", bufs=1, space="PSUM") as vpsp,
                tc.tile_pool(name="drains", bufs=2) as drainp,
                tc.tile_pool(name="ropetmp", bufs=2) as ropep,
            ):
                wq_sb = wsb.tile([128, DCH * HPC * HD], f16)
                wk_sb = wsb.tile([128, DCH * HPC * HD], f16)
                wv_sb = wsb.tile([128, DCH * HPC * HD], f16)
                # interleave by chunk so the first matmuls start early
                for d in range(DCH):
                    for t_sb, t_dram in ((wq_sb, wq), (wk_sb, wk), (wv_sb, wv)):
                        nc.sync.dma_start(
                            out=t_sb[:, d * HPC * HD:(d + 1) * HPC * HD],
                            in_=t_dram[d * 128:(d + 1) * 128, :],
                        )

                def rope(dst, dst_col, src, s0):
                    # src: [128, 512] sbuf fp16; rows 0:64 even dims, 64:128 odd
                    te = src[0:64, :]
                    to = src[64:128, :]
                    cl = cos_sb[0:64, s0:s0 + 512]
                    ch = cos_sb[64:128, s0:s0 + 512]
                    sl = sin_sb[0:64, s0:s0 + 512]
                    sh = sin_sb[64:128, s0:s0 + 512]
                    t1 = ropep.tile([64, 512], f16, tag="r1")
                    t2 = ropep.tile([64, 512], f16, tag="r2")
                    nc.vector.tensor_mul(t1[:], te, cl)
                    nc.vector.tensor_mul(t2[:], to, sh)
                    nc.vector.tensor_sub(dst[0:64, dst_col:dst_col + 512],
                                         t1[:], t2[:])
                    t3 = ropep.tile([64, 512], f16, tag="r3")
                    t4 = ropep.tile([64, 512], f16, tag="r4")
                    nc.vector.tensor_mul(t3[:], te, sl)
                    nc.vector.tensor_mul(t4[:], to, ch)
                    nc.vector.tensor_add(dst[64:128, dst_col:dst_col + 512],
                                         t3[:], t4[:])

                for rt in range(NRT):
                    s0 = (rt % NQT) * 512
                    qk = [qkps.tile([128, 512], f32, tag=f"qk{i}",
                                    name="qktile") for i in range(4)]
                    vp = [vpsp.tile([128, 256], f32, tag=f"vp{i}",
                                    name="vptile") for i in range(4)]
                    for d in range(DCH):
                        x_t = xpool.tile([128, 512], f16, tag="xt")
                        nc.gpsimd.dma_start(
                            out=x_t[:],
                            in_=xT[d * 128:(d + 1) * 128,
                                   rt * 512:(rt + 1) * 512],
                        )
                        first, last = d == 0, d == DCH - 1
                        c0 = d * HPC * HD
                        for hh in range(HPC):
                            nc.tensor.matmul(
                                qk[hh][:],
                                lhsT=wq_sb[:, c0 + hh * HD:c0 + (hh + 1) * HD],
                                rhs=x_t[:], start=first, stop=last,
                            )
                            nc.tensor.matmul(
                                qk[2 + hh][:],
                                lhsT=wk_sb[:, c0 + hh * HD:c0 + (hh + 1) * HD],
                                rhs=x_t[:], start=first, stop=last,
                            )
                        for sub in range(4):
                            nc.tensor.matmul(
                                vp[sub][:],
                                lhsT=x_t[:, sub * 128:(sub + 1) * 128],
                                rhs=wv_sb[:, c0:c0 + HPC * HD],
                                start=first, stop=last,
                            )
                    # fast drain: psum -> sbuf fp16 copies, then RoPE off-psum
                    for i in range(4):
                        qksb = drainp.tile([128, 512], f16, tag=f"dr{i}")
                        nc.scalar.copy(qksb[:], qk[i][:])
                        dst = qT if i < 2 else kT
                        rope(dst, (i % 2) * ROWS + rt * 512, qksb, s0)
                    for sub in range(4):
                        gc = rt * 4 + sub
                        nc.scalar.copy(
                            v_sb[:, gc * 256:(gc + 1) * 256], vp[sub][:])

            # ---- Attention per (head, batch), causal-skipped ----
            with tc.tile_pool(name="wop", bufs=1) as wop:
                # Wo resident, chunk-major (prefetched during attention)
                wo_sb = wop.tile([128, DCH * D], f16)
                for k in range(DCH):
                    nc.sync.dma_start(
                        out=wo_sb[:, k * D:(k + 1) * D],
                        in_=wo[k * 128:(k + 1) * 128, :],
                    )
                with (
                    tc.tile_pool(name="scps", bufs=4, space="PSUM") as scps,
                    tc.tile_pool(name="attps", bufs=1, space="PSUM") as attps,
                    tc.tile_pool(name="extiles", bufs=6) as expool,
                    tc.tile_pool(name="esum", bufs=2) as esump,
                    tc.tile_pool(name="araw", bufs=2) as arawp,
                    tc.tile_pool(name="tmp", bufs=2) as tmpp,
                ):
                    # deferred work from the previous head-batch, issued
                    # early in the next one so it never stalls an engine
                    pend_norm = []
                    pend_a2a = []

                    def flush_deferred():
                        for fn in pend_norm:
                            fn()
                        pend_norm.clear()
                        for fn in pend_a2a:
                            fn()
                        pend_a2a.clear()

                    for h in range(HPC):
                        for b in range(B):
                            kcol = h * ROWS + b * S
                            att = [attps.tile([128, 512], f32, tag=f"att{qt}",
                                              name="atttile")
                                   for qt in range(NQT)]
                            esum = [esump.tile([128, 512], f16, tag=f"es{qt}",
                                               name="esumtile")
                                    for qt in range(NQT)]
                            pend = []

                            def flush_attv(limit, att=att, h=h, b=b):
                                while len(pend) > limit:
                                    qt2, kc2, ex2 = pend.pop(0)
                                    gc2 = (b * DCH + kc2) * HPC * HD + h * HD
                                    nc.tensor.matmul(
                                        att[qt2][:],
                                        lhsT=v_sb[:, gc2:gc2 + HD],
                                        rhs=ex2[:],
                                        start=(kc2 == 0),
                                        stop=(kc2 == 4 * qt2 + 3),
                                    )

                            for kc in range(DCH):
                                for qt in range(kc // 4, NQT):
                                    diag = kc // 4 == qt
                                    sc = scps.tile([128, 512], f32, tag="sc")
                                    nc.tensor.matmul(
                                        sc[:],
                                        lhsT=kT[:, kcol + kc * 128:
                                                kcol + (kc + 1) * 128],
                                        rhs=qT[:, kcol + qt * 512:
                                               kcol + (qt + 1) * 512],
                                        start=True, stop=not diag,
                                    )
                                    if diag:  # mask on the PE: sc += I.T @ M
                                        kk = kc % 4
                                        nc.tensor.matmul(
                                            sc[:], lhsT=ident_sb[:],
                                            rhs=mask_sb[:, kk * 512:
                                                        (kk + 1) * 512],
                                            start=False, stop=True,
                                        )
                                    ex = expool.tile([128, 512], f16, tag="ex")
                                    nc.scalar.activation(ex[:], sc[:], Exp,
                                                         bias=eshift_sb[:],
                                                         scale=SCALE)
                                    if kc == 0:
                                        nc.vector.tensor_copy(esum[qt][:], ex[:])
                                    else:
                                        nc.vector.tensor_add(
                                            esum[qt][:], esum[qt][:], ex[:])
                                    pend.append((qt, kc, ex))
                                    flush_attv(3)
                                if kc == 1:
                                    flush_deferred()
                            flush_attv(0)
                            # drain att psum to sbuf fp16 (frees banks), then
                            # defer den/recip/normalize into the next batch
                            araw = [arawp.tile([128, 512], f16, tag=f"ar{qt}",
                                               name="arawtile")
                                    for qt in range(NQT)]
                            for qt in range(NQT):
                                nc.vector.tensor_copy(araw[qt][:], att[qt][:])

                            def norm(h=h, b=b, kcol=kcol, araw=araw,
                                     esum=esum):
                                for qt in range(NQT):
                                    den = scps.tile([128, 512], f32, tag="sc")
                                    nc.tensor.matmul(
                                        den[0:1, :], lhsT=ones_sb[:, 0:1],
                                        rhs=esum[qt][:], start=True, stop=True,
                                    )
                                    rcp32 = tmpp.tile([1, 512], f32,
                                                      tag="rcp32")
                                    nc.vector.reciprocal_approx_fast(
                                        rcp32[:], den[0:1, :])
                                    rcp = tmpp.tile([1, 512], f16, tag="rcp")
                                    nc.vector.tensor_copy(rcp[:], rcp32[:])
                                    rb = scps.tile([128, 512], f32, tag="sc")
                                    nc.tensor.matmul(
                                        rb[:], lhsT=ones_sb[0:1, :],
                                        rhs=rcp[:], start=True, stop=True,
                                    )
                                    rbs = tmpp.tile([128, 512], f16, tag="rbs")
                                    nc.vector.tensor_copy(rbs[:], rb[:])
                                    nc.vector.tensor_mul(
                                        attT[:, kcol + qt * 512:
                                             kcol + (qt + 1) * 512],
                                        araw[qt][:], rbs[:],
                                    )

                            pend_norm.append(norm)
                            if b == B - 1:
                                def a2a(h=h):
                                    for dd in range(NCORES):
                                        nc.gpsimd.dma_start(
                                            out=a2a_in[h][dd * 128:
                                                          (dd + 1) * 128, :],
                                            in_=attT[:, h * ROWS + dd * 512:
                                                     h * ROWS +
                                                     (dd + 1) * 512],
                                        )
                                    nc.gpsimd.collective_compute(
                                        "AllToAll",
                                        mybir.AluOpType.bypass,
                                        replica_groups=[list(range(NCORES))],
                                        ins=[a2a_in[h][:].opt()],
                                        outs=[a2a_out[h][:].opt()],
                                    )
                                pend_a2a.append(a2a)
                    flush_deferred()

                # ---- Output projection over exchange chunks ----
                with (
                    tc.tile_pool(name="atile", bufs=1) as atp,
                    tc.tile_pool(name="ops", bufs=2, space="PSUM") as opsp,
                    tc.tile_pool(name="osb", bufs=1) as osbp,
                ):
                    aT = atp.tile([128, DCH * 512], f16)
                    for k in range(DCH):
                        h, r = k // NCORES, k % NCORES
                        nc.gpsimd.dma_start(
                            out=aT[:, k * 512:(k + 1) * 512],
                            in_=a2a_out[h][r * 128:(r + 1) * 128, :],
                        )
                    # pass 1: head-A chunks (k<8) right after exchange A,
                    # overlapping exchange B's flight; pass 2 accumulates
                    # the head-B chunks and adds the partial sums.
                    osb1 = [osbp.tile([128, D], f32, tag=f"o1{m}",
                                      name="osb1tile") for m in range(4)]
                    for m in range(4):
                        ops = opsp.tile([128, D], f32, tag="ops",
                                        name="opstile")
                        for k in range(8):
                            for n in range(4):
                                nc.tensor.matmul(
                                    ops[:, n * 512:(n + 1) * 512],
                                    lhsT=aT[:, k * 512 + m * 128:
                                            k * 512 + (m + 1) * 128],
                                    rhs=wo_sb[:, k * D + n * 512:
                                              k * D + (n + 1) * 512],
                                    start=(k == 0), stop=(k == 7),
                                )
                        nc.scalar.copy(osb1[m][:], ops[:])
                    for m in range(4):
                        ops = opsp.tile([128, D], f32, tag="ops",
                                        name="opstile")
                        for k in range(8, DCH):
                            for n in range(4):
                                nc.tensor.matmul(
                                    ops[:, n * 512:(n + 1) * 512],
                                    lhsT=aT[:, k * 512 + m * 128:
                                            k * 512 + (m + 1) * 128],
                                    rhs=wo_sb[:, k * D + n * 512:
                                              k * D + (n + 1) * 512],
                                    start=(k == 8), stop=(k == DCH - 1),
                                )
                        osb = osbp.tile([128, D], f32, tag="osb")
                        nc.vector.tensor_add(osb[:], ops[:], osb1[m][:])
                        nc.sync.dma_start(
                            out=out[m * 128:(m + 1) * 128, :], in_=osb[:],
                        )

    nc.compile()
    return nc


def _get_graph():
    global _GRAPH
    if _GRAPH is None:
        _GRAPH = _build_graph()
    return _GRAPH


_EVENODD = np.concatenate([np.arange(0, HD, 2), np.arange(1, HD, 2)])


def kernel(x, Wq, Wk, Wv, Wo, freqs_cos, freqs_sin, mask):
    global _LAST_EXEC_NS, _LAST_RES
    from concourse.bass_utils import run_bass_kernel_spmd

    nc = _get_graph()

    x = np.asarray(x, np.float32)
    Wq = np.asarray(Wq, np.float32)
    Wk = np.asarray(Wk, np.float32)
    Wv = np.asarray(Wv, np.float32)
    Wo = np.asarray(Wo, np.float32)

    xTb = np.ascontiguousarray(x.reshape(ROWS, D).T).astype(F16)
    cosf = np.asarray(freqs_cos, np.float32).T.astype(F16)
    sinf = np.asarray(freqs_sin, np.float32).T.astype(F16)
    cosf = np.ascontiguousarray(np.concatenate([cosf, cosf], axis=0))
    sinf = np.ascontiguousarray(np.concatenate([sinf, sinf], axis=0))

    # 4 relative diagonal mask blocks; -30000 acts as -inf after scaling
    i_idx = np.arange(128)[:, None]
    q_idx = np.arange(512)[None, :]
    maskd = np.concatenate(
        [np.where(kk * 128 + i_idx > q_idx, np.float32(-30000.0),
                  np.float32(0.0)) for kk in range(4)], axis=1)
    maskd = np.ascontiguousarray(maskd).astype(F16)
    identh = np.eye(128, dtype=F16)

    # Wo rows permuted to AllToAll arrival order: heads 0,2,..,14,1,3,..,15
    row_order = np.concatenate(
        [np.arange(h * HD, (h + 1) * HD)
         for h in [2 * r for r in range(NCORES)]
         + [2 * r + 1 for r in range(NCORES)]])
    wo_p = np.ascontiguousarray(Wo[row_order, :]).astype(F16)
    ones_h = np.ones((128, 128), F16)

    in_maps = []
    for c in range(NCORES):
        cols_pq = np.concatenate(
            [(2 * c + hh) * HD + _EVENODD for hh in range(HPC)])
        cols_v = np.arange(2 * c * HD, (2 * c + HPC) * HD)
        in_maps.append({
            "xT": xTb,
            "wq": np.ascontiguousarray(Wq[:, cols_pq]).astype(F16),
            "wk": np.ascontiguousarray(Wk[:, cols_pq]).astype(F16),
            "wv": np.ascontiguousarray(Wv[:, cols_v]).astype(F16),
            "wo": wo_p,
            "cosT": cosf, "sinT": sinf,
            "maskd": maskd, "onesh": ones_h, "identh": identh,
        })

    res = run_bass_kernel_spmd(
        nc, in_maps, core_ids=list(range(NCORES)), trace=_TRACE,
    )
    _LAST_EXEC_NS = res.exec_time_ns
    _LAST_RES = res

    outp = np.empty((ROWS, D), np.float32)
    for c in range(NCORES):
        outp[c * ORON:(c + 1) * ORON, :] = res.results[c]["out"]
    return outp.reshape(B, S, D)
